# revision 17
# baseline (speedup 1.0000x reference)
"""Trainium2 Bass kernel for nn_MultiHeadAttention_76158360092785.

Multi-head self-attention (B=8, N=2048, C=256, H=8, dk=dv=64) + concat +
1x1 conv + BatchNorm(inference) + LeakyReLU.

Sharding: data-parallel over batch — core i processes batch element i.

Per-core dataflow (all matmul operands bf16, fp32 PSUM accumulation):
  - host pre-transposes x -> xT [C, N] and weights; BN is folded into the
    output projection (scale) and a bias tile (shift) on host.
  - projections compute qT/kT in a "transposed scores" layout so the
    softmax axis lands on the PSUM free axis never needing a transpose:
      qT[h*64+k, n], kT[h*64+k, m]   (2 heads per 128-partition tile)
      v[m, h*65+v] with a ones column appended per head (row-sum trick)
  - per head h: stage duplicated qT2/kT2 [128, N] (both partition halves
    hold head h) via SBUF->SBUF DMA so K=64 energy matmuls can be packed
    2x into PE row groups (0,0)/(64,0).
  - m-loop (16 chunks of 128): S^T[m_chunk, :] = kT2.T @ qT2 (4 matmuls,
    N=512, into a 4-bank PSUM tensor [128, 2048]); one ACT instruction
    exp(0.125 * S^T) -> PT bf16 in SBUF; 4 attnV matmuls
    [V_h|1].T @ PT accumulate into PSUM [65, 2048] (row 64 = softmax
    denominators).
  - epilogue: DMA-broadcast the denominator row, reciprocal, multiply ->
    catT[f, n] (f = h*64+v), then output projection y = catT.T @ WpT*s,
    + bias, LeakyReLU, DMA out.
"""

import numpy as np
import ml_dtypes

import concourse.bass as bass
import concourse.bacc as bacc
import concourse.tile as tile
import concourse.mybir as mybir
from concourse.alu_op_type import AluOpType

F32 = mybir.dt.float32
BF16 = mybir.dt.bfloat16

BN_EPS = 1e-5
LEAKY_SLOPE = 0.01


def build_kernel_body(tc, ins, outs, N=2048, C=256, H=8, DK=64, reps=1):
    """Emit the Tile program. ins/outs are dicts of DRAM APs.

    reps>1 wraps the whole body in a device-side For_i loop — used only
    for timing (the work is idempotent)."""
    if reps > 1:
        with tc.For_i(0, reps, 1):
            _kernel_body(tc, ins, outs, N, C, H, DK)
    else:
        _kernel_body(tc, ins, outs, N, C, H, DK)


def _kernel_body(tc, ins, outs, N, C, H, DK):
    nc = tc.nc
    MC = N // 128          # number of 128-row m/n chunks (16)
    NC4 = N // 512         # number of 512-wide n chunks (4)
    NH = max(1, NC4 // 2)  # 512-chunks per half-score tile (2)
    HW = NH * 512          # half-score tile width (1024)
    CC = C // 128          # c chunks (2)
    HT = H // 2            # two heads per 128-partition tile (4)
    F = H * DK             # 512
    FC = F // 128          # f chunks for out-proj (4)
    SCALE = 1.0 / float(np.sqrt(DK))

    xT_d = ins["xT"]          # [C, N] bf16   (x[b].T)
    wqT_d = ins["wqT"]        # [C, F] bf16   (Wq.reshape(F,C).T)
    wkT_d = ins["wkT"]        # [C, F] bf16
    wvT_d = ins["wvT"]        # [C, F] bf16
    wpTs_d = ins["wpTs"]      # [F, C] bf16   ((Wp*s[:,None]).T)
    brep_d = ins["b_rep"]     # [128, C] f32  (beta - mean*s, replicated)
    y_d = outs["y"]           # [N, C] f32

    import contextlib
    with contextlib.ExitStack() as ctx:
        persist = ctx.enter_context(tc.tile_pool(name="persist", bufs=1))
        work = ctx.enter_context(tc.tile_pool(name="work", bufs=2))
        ptpool = ctx.enter_context(tc.tile_pool(name="ptp", bufs=4))
        stage = ctx.enter_context(tc.tile_pool(name="stage", bufs=2))

        # ---------------- phase 0: load inputs ----------------
        xT = persist.tile([128, CC, N], BF16, tag="xT")
        for ci in range(CC):
            nc.sync.dma_start(out=xT[:, ci, :], in_=xT_d[ci * 128:(ci + 1) * 128, :])
        wq = persist.tile([128, CC, F], BF16, tag="wq")
        wk = persist.tile([128, CC, F], BF16, tag="wk")
        wv = persist.tile([128, CC, F], BF16, tag="wv")
        for (w_t, w_d) in ((wq, wqT_d), (wk, wkT_d), (wv, wvT_d)):
            for ci in range(CC):
                nc.sync.dma_start(out=w_t[:, ci, :], in_=w_d[ci * 128:(ci + 1) * 128, :])
        wp = persist.tile([128, FC, C], BF16, tag="wp")
        for fi in range(FC):
            nc.sync.dma_start(out=wp[:, fi, :], in_=wpTs_d[fi * 128:(fi + 1) * 128, :])
        brep = persist.tile([128, C], F32, tag="brep")
        nc.sync.dma_start(out=brep, in_=brep_d)

        # persistent stores
        qT = persist.tile([128, HT, N], BF16, tag="qT")   # [h%2*64+k, ht, n]
        kT = persist.tile([128, HT, N], BF16, tag="kT")
        # v3: per (m-chunk, head): [V_h | ones] as one contiguous 128-col
        # block so the attnV weights AP has a single free dim; out rows
        # 0-63 = xw, rows 64-127 = softmax row sums (replicated).
        v3 = persist.tile([128, MC, H, 128], BF16, tag="v3")
        catT = persist.tile([128, FC, N], BF16, tag="catT")

        nc.vector.memset(v3[:, :, :, 64:128], 1.0)

        # psum pools: stA = 2x [128,1024] (4 banks), acc = [128,2048] (4 banks)
        with tc.tile_pool(name="ps_a", bufs=2, space="PSUM") as ps_a, \
             tc.tile_pool(name="ps_b", bufs=1, space="PSUM") as ps_b:

            # ---------------- phase 1: projections (compact) ----------------
            # qT/kT: out[M=128 (2 heads' dk), N=n] ; lhsT = wqT chunk [c, 128]
            for (w_t, dst) in ((wq, qT), (wk, kT)):
                for t in range(HT):
                    for half in range(2):
                        pj = ps_a.tile([128, HW], F32, tag="stA")
                        for j2 in range(NH):
                            j = half * NH + j2
                            for ci in range(CC):
                                nc.tensor.matmul(
                                    pj[:, j2 * 512:(j2 + 1) * 512],
                                    w_t[:, ci, t * 128:(t + 1) * 128],
                                    xT[:, ci, j * 512:(j + 1) * 512],
                                    start=(ci == 0), stop=(ci == CC - 1),
                                )
                        nc.vector.tensor_copy(
                            out=dst[:, t, half * HW:(half + 1) * HW], in_=pj)
            # v: out[M=m chunk, N=f] ; lhsT = xT chunk [c, m], rhs = wvT [c, f]
            for mi in range(MC):
                pj = ps_a.tile([128, HW], F32, tag="stA")
                for ci in range(CC):
                    nc.tensor.matmul(
                        pj[:, 0:F],
                        xT[:, ci, mi * 128:(mi + 1) * 128],
                        wv[:, ci, :],
                        start=(ci == 0), stop=(ci == CC - 1),
                    )
                nc.vector.tensor_copy(
                    out=v3[:, mi, :, 0:64],
                    in_=pj[:, 0:F].rearrange("p (h w) -> p h w", w=64),
                )

            # ---------------- phase 2: attention per head ----------------
            for h in range(H):
                t, half = h // 2, (h % 2) * 64
                # duplicated staging: both partition halves hold head h
                q2 = stage.tile([128, N], BF16, tag="q2")
                k2 = stage.tile([128, N], BF16, tag="k2")
                for (dst, src) in ((q2, qT), (k2, kT)):
                    for g in range(2):
                        nc.sync.dma_start(
                            out=dst[g * 64:(g + 1) * 64, :],
                            in_=src[half:half + 64, t, :],
                        )

                acc = ps_b.tile([128, N], F32, tag="ac")

                def attn_v(mi):
                    for j in range(NC4):
                        nc.tensor.matmul(
                            acc[:, j * 512:(j + 1) * 512],
                            v3[:, mi, h, :],
                            pts[mi % 4][:, j * 512:(j + 1) * 512],
                            start=(mi == 0), stop=(mi == MC - 1),
                            skip_group_check=True,
                        )

                pts = [None, None, None, None]
                for mi in range(MC):
                    sta = ps_a.tile([128, HW], F32, tag="stA")
                    for j in range(NH):
                        g = (j % 2) * 64
                        nc.tensor.matmul(
                            sta[:, j * 512:(j + 1) * 512],
                            k2[g:g + 64, mi * 128:(mi + 1) * 128],
                            q2[g:g + 64, j * 512:(j + 1) * 512],
                            start=True, stop=True,
                        )
                    stb = ps_a.tile([128, HW], F32, tag="stA")
                    for j in range(NH, 2 * NH):
                        g = (j % 2) * 64
                        nc.tensor.matmul(
                            stb[:, (j - NH) * 512:(j - NH + 1) * 512],
                            k2[g:g + 64, mi * 128:(mi + 1) * 128],
                            q2[g:g + 64, j * 512:(j + 1) * 512],
                            start=True, stop=True,
                        )
                    # attnV runs two chunks behind so PE never gates ACT
                    if mi >= 2:
                        attn_v(mi - 2)
                    pt = ptpool.tile([128, N], BF16, tag="pt")
                    nc.scalar.activation(out=pt[:, 0:HW], in_=sta,
                                         func=mybir.ActivationFunctionType.Exp,
                                         scale=SCALE)
                    nc.scalar.activation(out=pt[:, HW:2 * HW], in_=stb,
                                         func=mybir.ActivationFunctionType.Exp,
                                         scale=SCALE)
                    pts[mi % 4] = pt
                attn_v(MC - 2)
                attn_v(MC - 1)

                # epilogue: row sums are replicated on partitions 64..127.
                # Processed in two n-halves so the recip->move->mul chain
                # pipelines (halves the head-boundary latency).
                rec = work.tile([128, N], F32, tag="rec")
                rlo = work.tile([64, N], F32, tag="rlo")
                xwn = work.tile([64, N], BF16, tag="xwn")
                for u in range(2):
                    s = slice(u * (N // 2), (u + 1) * (N // 2))
                    nc.vector.reciprocal(out=rec[64:128, s], in_=acc[64:128, s])
                    nc.sync.dma_start(out=rlo[:, s], in_=rec[64:128, s])
                    nc.vector.tensor_tensor(out=xwn[:, s], in0=acc[0:64, s],
                                            in1=rlo[:, s], op=AluOpType.mult)
                    nc.sync.dma_start(out=catT[half:half + 64, t, s],
                                      in_=xwn[:, s])

        # ---------------- phase 3: output projection ----------------
        with tc.tile_pool(name="ps_y", bufs=4, space="PSUM") as ps_y:
            for nt in range(MC):
                yb = ps_y.tile([128, C], F32, tag="yb")
                for fi in range(FC):
                    nc.tensor.matmul(
                        yb,
                        catT[:, fi, nt * 128:(nt + 1) * 128],
                        wp[:, fi, :],
                        start=(fi == 0), stop=(fi == FC - 1),
                    )
                yt = work.tile([128, C], F32, tag="yt")
                nc.vector.tensor_tensor(out=yt, in0=yb, in1=brep, op=AluOpType.add)
                yo = work.tile([128, C], F32, tag="yo")
                nc.vector.scalar_tensor_tensor(out=yo, in0=yt, scalar=LEAKY_SLOPE,
                                               in1=yt, op0=AluOpType.mult,
                                               op1=AluOpType.max)
                nc.sync.dma_start(out=y_d[nt * 128:(nt + 1) * 128, :], in_=yo)


def prep_inputs(x, Wq, Wk, Wv, Wp, bn_gamma, bn_beta, bn_mean, bn_var):
    """Host-side preprocessing: transposes, BN folding, bf16 casts.
    Returns per-core in_maps (core i = batch element i)."""
    B, N, C = x.shape
    H, dk, _ = Wq.shape
    F = H * dk
    s = (bn_gamma / np.sqrt(bn_var + BN_EPS)).astype(np.float32)
    bias = (bn_beta - bn_mean * s).astype(np.float32)
    b_rep = np.ascontiguousarray(np.broadcast_to(bias, (128, C)))

    bf = ml_dtypes.bfloat16
    wqT = np.ascontiguousarray(Wq.reshape(F, C).T).astype(bf)
    wkT = np.ascontiguousarray(Wk.reshape(F, C).T).astype(bf)
    wvT = np.ascontiguousarray(Wv.reshape(F, C).T).astype(bf)
    wpTs = np.ascontiguousarray((Wp * s[:, None]).T).astype(bf)

    in_maps = []
    for b in range(B):
        in_maps.append({
            "xT": np.ascontiguousarray(x[b].T).astype(bf),
            "wqT": wqT, "wkT": wkT, "wvT": wvT, "wpTs": wpTs,
            "b_rep": b_rep,
        })
    return in_maps


_CACHE = {}


def _build_program(N=2048, C=256, H=8, DK=64, reps=1):
    key = (N, C, H, DK, reps)
    if key in _CACHE:
        return _CACHE[key]
    F = H * DK
    nc = bacc.Bacc("TRN2", target_bir_lowering=False, debug=False)
    ins = {
        "xT": nc.dram_tensor("xT", (C, N), BF16, kind="ExternalInput").ap(),
        "wqT": nc.dram_tensor("wqT", (C, F), BF16, kind="ExternalInput").ap(),
        "wkT": nc.dram_tensor("wkT", (C, F), BF16, kind="ExternalInput").ap(),
        "wvT": nc.dram_tensor("wvT", (C, F), BF16, kind="ExternalInput").ap(),
        "wpTs": nc.dram_tensor("wpTs", (F, C), BF16, kind="ExternalInput").ap(),
        "b_rep": nc.dram_tensor("b_rep", (128, C), F32, kind="ExternalInput").ap(),
    }
    outs = {"y": nc.dram_tensor("y", (N, C), F32, kind="ExternalOutput").ap()}
    with tile.TileContext(nc) as tc:
        build_kernel_body(tc, ins, outs, N=N, C=C, H=H, DK=DK, reps=reps)
    nc.compile()
    _CACHE[key] = nc
    return nc


def kernel(x, Wq, Wk, Wv, Wp, bn_gamma, bn_beta, bn_mean, bn_var):
    from concourse.bass_utils import run_bass_kernel_spmd
    x = np.asarray(x, dtype=np.float32)
    in_maps = prep_inputs(np.asarray(x, np.float32), np.asarray(Wq, np.float32),
                          np.asarray(Wk, np.float32), np.asarray(Wv, np.float32),
                          np.asarray(Wp, np.float32), np.asarray(bn_gamma, np.float32),
                          np.asarray(bn_beta, np.float32), np.asarray(bn_mean, np.float32),
                          np.asarray(bn_var, np.float32))
    B, N, C = x.shape
    nc = _build_program(N=N, C=C, H=Wq.shape[0], DK=Wq.shape[1])
    res = run_bass_kernel_spmd(nc, in_maps, core_ids=list(range(B)))
    y = np.stack([res.results[i]["y"] for i in range(B)]).astype(np.float32)
    return y


# revision 18
# speedup vs baseline: 1.1522x; 1.1522x over previous
"""Trainium2 Bass kernel for nn_MultiHeadAttention_76158360092785.

Multi-head self-attention (B=8, N=2048, C=256, H=8, dk=dv=64) + concat +
1x1 conv + BatchNorm(inference) + LeakyReLU.

Sharding: data-parallel over batch — core i processes batch element i.

Per-core dataflow (all matmul operands bf16, fp32 PSUM accumulation):
  - host pre-transposes x -> xT [C, N] and weights; BN is folded into the
    output projection (scale) and a bias tile (shift) on host.
  - projections compute qT/kT in a "transposed scores" layout so the
    softmax axis lands on the PSUM free axis never needing a transpose:
      qT[h*64+k, n], kT[h*64+k, m]   (2 heads per 128-partition tile)
      v[m, h*65+v] with a ones column appended per head (row-sum trick)
  - per head h: stage duplicated qT2/kT2 [128, N] (both partition halves
    hold head h) via SBUF->SBUF DMA so K=64 energy matmuls can be packed
    2x into PE row groups (0,0)/(64,0).
  - m-loop (16 chunks of 128): S^T[m_chunk, :] = kT2.T @ qT2 (4 matmuls,
    N=512, into a 4-bank PSUM tensor [128, 2048]); one ACT instruction
    exp(0.125 * S^T) -> PT bf16 in SBUF; 4 attnV matmuls
    [V_h|1].T @ PT accumulate into PSUM [65, 2048] (row 64 = softmax
    denominators).
  - epilogue: DMA-broadcast the denominator row, reciprocal, multiply ->
    catT[f, n] (f = h*64+v), then output projection y = catT.T @ WpT*s,
    + bias, LeakyReLU, DMA out.
"""

import numpy as np
import ml_dtypes

import concourse.bass as bass
import concourse.bacc as bacc
import concourse.tile as tile
import concourse.mybir as mybir
from concourse.alu_op_type import AluOpType

F32 = mybir.dt.float32
BF16 = mybir.dt.bfloat16

BN_EPS = 1e-5
LEAKY_SLOPE = 0.01


def build_kernel_body(tc, ins, outs, N=2048, C=256, H=8, DK=64, reps=1):
    """Emit the Tile program. ins/outs are dicts of DRAM APs.

    reps>1 wraps the whole body in a device-side For_i loop — used only
    for timing (the work is idempotent)."""
    if reps > 1:
        with tc.For_i(0, reps, 1):
            _kernel_body(tc, ins, outs, N, C, H, DK)
    else:
        _kernel_body(tc, ins, outs, N, C, H, DK)


def _kernel_body(tc, ins, outs, N, C, H, DK):
    nc = tc.nc
    MC = N // 128          # number of 128-row m/n chunks (16)
    NC4 = N // 512         # number of 512-wide n chunks (4)
    NH = max(1, NC4 // 2)  # 512-chunks per half-score tile (2)
    HW = NH * 512          # half-score tile width (1024)
    CC = C // 128          # c chunks (2)
    HT = H // 2            # two heads per 128-partition tile (4)
    F = H * DK             # 512
    FC = F // 128          # f chunks for out-proj (4)
    SCALE = 1.0 / float(np.sqrt(DK))

    xT_d = ins["xT"]          # [C, N] bf16   (x[b].T)
    wqT_d = ins["wqT"]        # [C, F] bf16   (Wq.reshape(F,C).T)
    wkT_d = ins["wkT"]        # [C, F] bf16
    wvT_d = ins["wvT"]        # [C, F] bf16
    wpTs_d = ins["wpTs"]      # [F, C] bf16   ((Wp*s[:,None]).T)
    brep_d = ins["b_rep"]     # [128, C] f32  (beta - mean*s, replicated)
    y_d = outs["y"]           # [N, C] f32

    import contextlib
    with contextlib.ExitStack() as ctx:
        persist = ctx.enter_context(tc.tile_pool(name="persist", bufs=1))
        work = ctx.enter_context(tc.tile_pool(name="work", bufs=2))
        ptpool = ctx.enter_context(tc.tile_pool(name="ptp", bufs=4))
        stage = ctx.enter_context(tc.tile_pool(name="stage", bufs=2))

        # ---------------- phase 0: load inputs ----------------
        xT = persist.tile([128, CC, N], BF16, tag="xT")
        for ci in range(CC):
            nc.sync.dma_start(out=xT[:, ci, :], in_=xT_d[ci * 128:(ci + 1) * 128, :])
        wq = persist.tile([128, CC, F], BF16, tag="wq")
        wk = persist.tile([128, CC, F], BF16, tag="wk")
        wv = persist.tile([128, CC, F], BF16, tag="wv")
        for (w_t, w_d) in ((wq, wqT_d), (wk, wkT_d), (wv, wvT_d)):
            for ci in range(CC):
                nc.sync.dma_start(out=w_t[:, ci, :], in_=w_d[ci * 128:(ci + 1) * 128, :])
        wp = persist.tile([128, FC, C], BF16, tag="wp")
        for fi in range(FC):
            nc.sync.dma_start(out=wp[:, fi, :], in_=wpTs_d[fi * 128:(fi + 1) * 128, :])
        brep = persist.tile([128, C], F32, tag="brep")
        nc.sync.dma_start(out=brep, in_=brep_d)

        # persistent stores
        qT = persist.tile([128, HT, N], BF16, tag="qT")   # [h%2*64+k, ht, n]
        kT = persist.tile([128, HT, N], BF16, tag="kT")
        # v3: per (m-chunk, head): [V_h | ones] as one contiguous 128-col
        # block so the attnV weights AP has a single free dim; out rows
        # 0-63 = xw, rows 64-127 = softmax row sums (replicated).
        v3 = persist.tile([128, MC, H, 128], BF16, tag="v3")
        catT = persist.tile([128, FC, N], BF16, tag="catT")

        nc.vector.memset(v3[:, :, :, 64:128], 1.0)

        # psum pools: stA = 2x [128,1024] (4 banks), acc = [128,2048] (4 banks)
        with tc.tile_pool(name="ps_a", bufs=2, space="PSUM") as ps_a, \
             tc.tile_pool(name="ps_b", bufs=1, space="PSUM") as ps_b:

            # ---------------- phase 1: projections (compact) ----------------
            # qT/kT: out[M=128 (2 heads' dk), N=n] ; lhsT = wqT chunk [c, 128]
            for (w_t, dst) in ((wq, qT), (wk, kT)):
                for t in range(HT):
                    for half in range(2):
                        pj = ps_a.tile([128, HW], F32, tag="stA")
                        for j2 in range(NH):
                            j = half * NH + j2
                            for ci in range(CC):
                                nc.tensor.matmul(
                                    pj[:, j2 * 512:(j2 + 1) * 512],
                                    w_t[:, ci, t * 128:(t + 1) * 128],
                                    xT[:, ci, j * 512:(j + 1) * 512],
                                    start=(ci == 0), stop=(ci == CC - 1),
                                )
                        nc.vector.tensor_copy(
                            out=dst[:, t, half * HW:(half + 1) * HW], in_=pj)
            # v: out[M=m chunk, N=f] ; lhsT = xT chunk [c, m], rhs = wvT [c, f]
            for mi in range(MC):
                pj = ps_a.tile([128, HW], F32, tag="stA")
                for ci in range(CC):
                    nc.tensor.matmul(
                        pj[:, 0:F],
                        xT[:, ci, mi * 128:(mi + 1) * 128],
                        wv[:, ci, :],
                        start=(ci == 0), stop=(ci == CC - 1),
                    )
                nc.vector.tensor_copy(
                    out=v3[:, mi, :, 0:64],
                    in_=pj[:, 0:F].rearrange("p (h w) -> p h w", w=64),
                )

            # ---------------- phase 2: attention per head ----------------
            for h in range(H):
                t, half = h // 2, (h % 2) * 64
                # duplicated staging: both partition halves hold head h
                q2 = stage.tile([128, N], BF16, tag="q2")
                k2 = stage.tile([128, N], BF16, tag="k2")
                for (dst, src) in ((q2, qT), (k2, kT)):
                    for g in range(2):
                        nc.sync.dma_start(
                            out=dst[g * 64:(g + 1) * 64, :],
                            in_=src[half:half + 64, t, :],
                        )

                acc = ps_b.tile([128, N], F32, tag="ac")

                def attn_v(mi):
                    for j in range(NC4):
                        nc.tensor.matmul(
                            acc[:, j * 512:(j + 1) * 512],
                            v3[:, mi, h, :],
                            pts[mi % 4][:, j * 512:(j + 1) * 512],
                            start=(mi == 0), stop=(mi == MC - 1),
                            skip_group_check=True,
                        )

                pts = [None, None, None, None]
                for mi in range(MC):
                    sta = ps_a.tile([128, HW], F32, tag="stA")
                    for j in range(NH):
                        g = (j % 2) * 64
                        nc.tensor.matmul(
                            sta[:, j * 512:(j + 1) * 512],
                            k2[g:g + 64, mi * 128:(mi + 1) * 128],
                            q2[g:g + 64, j * 512:(j + 1) * 512],
                            start=True, stop=True,
                        )
                    stb = ps_a.tile([128, HW], F32, tag="stA")
                    for j in range(NH, 2 * NH):
                        g = (j % 2) * 64
                        nc.tensor.matmul(
                            stb[:, (j - NH) * 512:(j - NH + 1) * 512],
                            k2[g:g + 64, mi * 128:(mi + 1) * 128],
                            q2[g:g + 64, j * 512:(j + 1) * 512],
                            start=True, stop=True,
                        )
                    # attnV runs two chunks behind so PE never gates ACT
                    if mi >= 2:
                        attn_v(mi - 2)
                    pt = ptpool.tile([128, N], BF16, tag="pt")
                    nc.scalar.activation(out=pt[:, 0:HW], in_=sta,
                                         func=mybir.ActivationFunctionType.Exp,
                                         scale=SCALE)
                    nc.scalar.activation(out=pt[:, HW:2 * HW], in_=stb,
                                         func=mybir.ActivationFunctionType.Exp,
                                         scale=SCALE)
                    pts[mi % 4] = pt
                attn_v(MC - 2)
                attn_v(MC - 1)

                # epilogue: row sums are replicated on partitions 64..127.
                # Free `acc` ASAP (recip_fast + copy are the only PSUM
                # readers); the partition-move DMA + normalize + store then
                # overlap the next head's m-loop.
                rec = work.tile([128, N], F32, tag="rec", bufs=1)
                xwr = work.tile([64, N], F32, tag="xwr")
                rlo = work.tile([64, N], F32, tag="rlo")
                xwn = work.tile([64, N], BF16, tag="xwn")
                for u in range(2):
                    s = slice(u * (N // 2), (u + 1) * (N // 2))
                    nc.vector.reciprocal_approx_fast(out=rec[64:128, s],
                                                     in_=acc[64:128, s])
                    nc.vector.tensor_copy(out=xwr[:, s], in_=acc[0:64, s])
                for u in range(2):
                    s = slice(u * (N // 2), (u + 1) * (N // 2))
                    nc.sync.dma_start(out=rlo[:, s], in_=rec[64:128, s])
                    nc.vector.tensor_tensor(out=xwn[:, s], in0=xwr[:, s],
                                            in1=rlo[:, s], op=AluOpType.mult)
                    nc.sync.dma_start(out=catT[half:half + 64, t, s],
                                      in_=xwn[:, s])

        # ---------------- phase 3: output projection ----------------
        with tc.tile_pool(name="ps_y", bufs=4, space="PSUM") as ps_y:
            for nt in range(MC):
                yb = ps_y.tile([128, C], F32, tag="yb")
                for fi in range(FC):
                    nc.tensor.matmul(
                        yb,
                        catT[:, fi, nt * 128:(nt + 1) * 128],
                        wp[:, fi, :],
                        start=(fi == 0), stop=(fi == FC - 1),
                    )
                yt = work.tile([128, C], F32, tag="yt")
                nc.vector.tensor_tensor(out=yt, in0=yb, in1=brep, op=AluOpType.add)
                yo = work.tile([128, C], F32, tag="yo")
                nc.vector.scalar_tensor_tensor(out=yo, in0=yt, scalar=LEAKY_SLOPE,
                                               in1=yt, op0=AluOpType.mult,
                                               op1=AluOpType.max)
                nc.sync.dma_start(out=y_d[nt * 128:(nt + 1) * 128, :], in_=yo)


def prep_inputs(x, Wq, Wk, Wv, Wp, bn_gamma, bn_beta, bn_mean, bn_var):
    """Host-side preprocessing: transposes, BN folding, bf16 casts.
    Returns per-core in_maps (core i = batch element i)."""
    B, N, C = x.shape
    H, dk, _ = Wq.shape
    F = H * dk
    s = (bn_gamma / np.sqrt(bn_var + BN_EPS)).astype(np.float32)
    bias = (bn_beta - bn_mean * s).astype(np.float32)
    b_rep = np.ascontiguousarray(np.broadcast_to(bias, (128, C)))

    bf = ml_dtypes.bfloat16
    wqT = np.ascontiguousarray(Wq.reshape(F, C).T).astype(bf)
    wkT = np.ascontiguousarray(Wk.reshape(F, C).T).astype(bf)
    wvT = np.ascontiguousarray(Wv.reshape(F, C).T).astype(bf)
    wpTs = np.ascontiguousarray((Wp * s[:, None]).T).astype(bf)

    in_maps = []
    for b in range(B):
        in_maps.append({
            "xT": np.ascontiguousarray(x[b].T).astype(bf),
            "wqT": wqT, "wkT": wkT, "wvT": wvT, "wpTs": wpTs,
            "b_rep": b_rep,
        })
    return in_maps


_CACHE = {}


def _build_program(N=2048, C=256, H=8, DK=64, reps=1):
    key = (N, C, H, DK, reps)
    if key in _CACHE:
        return _CACHE[key]
    F = H * DK
    nc = bacc.Bacc("TRN2", target_bir_lowering=False, debug=False)
    ins = {
        "xT": nc.dram_tensor("xT", (C, N), BF16, kind="ExternalInput").ap(),
        "wqT": nc.dram_tensor("wqT", (C, F), BF16, kind="ExternalInput").ap(),
        "wkT": nc.dram_tensor("wkT", (C, F), BF16, kind="ExternalInput").ap(),
        "wvT": nc.dram_tensor("wvT", (C, F), BF16, kind="ExternalInput").ap(),
        "wpTs": nc.dram_tensor("wpTs", (F, C), BF16, kind="ExternalInput").ap(),
        "b_rep": nc.dram_tensor("b_rep", (128, C), F32, kind="ExternalInput").ap(),
    }
    outs = {"y": nc.dram_tensor("y", (N, C), F32, kind="ExternalOutput").ap()}
    with tile.TileContext(nc) as tc:
        build_kernel_body(tc, ins, outs, N=N, C=C, H=H, DK=DK, reps=reps)
    nc.compile()
    _CACHE[key] = nc
    return nc


def kernel(x, Wq, Wk, Wv, Wp, bn_gamma, bn_beta, bn_mean, bn_var):
    from concourse.bass_utils import run_bass_kernel_spmd
    x = np.asarray(x, dtype=np.float32)
    in_maps = prep_inputs(np.asarray(x, np.float32), np.asarray(Wq, np.float32),
                          np.asarray(Wk, np.float32), np.asarray(Wv, np.float32),
                          np.asarray(Wp, np.float32), np.asarray(bn_gamma, np.float32),
                          np.asarray(bn_beta, np.float32), np.asarray(bn_mean, np.float32),
                          np.asarray(bn_var, np.float32))
    B, N, C = x.shape
    nc = _build_program(N=N, C=C, H=Wq.shape[0], DK=Wq.shape[1])
    res = run_bass_kernel_spmd(nc, in_maps, core_ids=list(range(B)))
    y = np.stack([res.results[i]["y"] for i in range(B)]).astype(np.float32)
    return y
